# revision 12
# baseline (speedup 1.0000x reference)
"""Trainium2 Bass kernel for nn_GatGraphClassifier (2-layer dense GAT + mean-pool classifier).

Sharding: 8 cores = 4 graphs x 2 head-groups.
  - core c handles graph b = c // 2 and layer-2 head group g = c % 2 (4 of 8 heads).
  - layer 1 (all 8 heads) is computed redundantly by both cores of a graph pair.
  - per-core device output: gpart[1, F] = sum_i sum_{h in grp} (attn2_h @ p2_h + x1 @ skip2_h)[i, :]
  - host combines: logits_b = ((gpart_2b + gpart_2b+1) / (8*N) + b2) @ Wc + bc

Math notes:
  - scores kept in [j (source node, partition), i (target node, free)] layout so that
    128x128 blocks of A = exp(leaky(E)) are directly usable as matmul lhsT.
  - exp(leaky_relu(E, 0.2)) == max(exp(E), exp(0.2*E)) (exp is monotone), so no
    explicit leaky pass: two ACT Exp passes (scale=0.2 for the second, per-partition
    s_tgt bias) + one elementwise max.
  - softmax row sums come for free as a 257th ones-column of the attention matmul rhs.
  - elu(z) = relu(z) + exp(min(z,0)) - 1, computed as r=Relu(z); z-=r; out=(Exp(z)-1)+r.
  - b1 is folded into the skip matmul as an extra contraction row (ones row in xT,
    b1 row in w1s).
"""

import numpy as np
import ml_dtypes

import concourse.bass as bass
import concourse.tile as tile
from concourse import bacc, mybir
from concourse.bass_utils import run_bass_kernel_spmd

dt = mybir.dt
AF = mybir.ActivationFunctionType
ALU = mybir.AluOpType
BF16 = ml_dtypes.bfloat16

# Problem dims (hardcoded per contract)
B, N, H, F, DIN, NCLS = 4, 1024, 8, 256, 256, 10
NCORES = 8
G = H // 2          # heads per core in layer 2
C1 = H * F          # 2048, layer-1 output width
P = 128

CFG = dict(n=N, h1=H, g2=G, f=F, din=DIN)


def build_program(cfg=None, has_mask=False):
    """Build + compile the SPMD single-core program. Returns compiled Bacc."""
    cfg = dict(CFG if cfg is None else cfg)
    n, h1, g2, f, din = cfg["n"], cfg["h1"], cfg["g2"], cfg["f"], cfg["din"]
    c1 = h1 * f
    w2cols = g2 * f + f + 2 * g2   # [W2grp | skip2sum | wsrc2 | wtgt2]

    nc = bacc.Bacc("TRN2", target_bir_lowering=False, debug=False,
                   num_devices=NCORES)

    d = {}
    d["xT"] = nc.dram_tensor("xT", [din + 1, n], dt.bfloat16, kind="ExternalInput").ap()
    d["w1p"] = nc.dram_tensor("w1p", [din, c1 + 2 * h1], dt.bfloat16, kind="ExternalInput").ap()
    d["w1s"] = nc.dram_tensor("w1s", [din + 1, c1], dt.bfloat16, kind="ExternalInput").ap()
    d["w2e"] = nc.dram_tensor("w2e", [c1, w2cols], dt.bfloat16, kind="ExternalInput").ap()
    d["idf"] = nc.dram_tensor("idf", [P, P], dt.float32, kind="ExternalInput").ap()
    d["idb"] = nc.dram_tensor("idb", [P, P], dt.bfloat16, kind="ExternalInput").ap()
    if has_mask:
        d["expmT"] = nc.dram_tensor("expmT", [n, n], dt.bfloat16, kind="ExternalInput").ap()
    d["gpart"] = nc.dram_tensor("gpart", [1, f], dt.float32, kind="ExternalOutput").ap()

    with tile.TileContext(nc) as tc:
        _emit(tc, cfg, has_mask, d)
    nc.compile()
    return nc


def _emit(tc, cfg, has_mask, d):
    nc = tc.nc
    n, h1, g2, f, din = cfg["n"], cfg["h1"], cfg["g2"], cfg["f"], cfg["din"]
    c1 = h1 * f
    nb = n // P
    kt1 = din // P
    kt2 = c1 // P
    ct = c1 // P
    fp1 = f + 1
    w2cols = g2 * f + f + 2 * g2
    em_d = d.get("expmT")

    with tc.tile_pool(name="pp", bufs=1) as pp:
        # ---- persistent tiles ----
        xT = []
        for k in range(kt1):
            t = pp.tile([P, n], dt.bfloat16, tag=f"xT{k}", name=f"xT{k}")
            nc.sync.dma_start(t[:], d["xT"][k * P:(k + 1) * P, :])
            xT.append(t)
        xTones = pp.tile([1, n], dt.bfloat16, tag="xTones", name="xTones")
        nc.sync.dma_start(xTones[:], d["xT"][din:din + 1, :])
        idf = pp.tile([P, P], dt.float32, tag="idf", name="idf")
        nc.sync.dma_start(idf[:], d["idf"][:])
        idb = pp.tile([P, P], dt.bfloat16, tag="idb", name="idb")
        nc.sync.dma_start(idb[:], d["idb"][:])
        sT1 = pp.tile([2 * h1, n], dt.float32, tag="sT1", name="sT1")
        sT2 = pp.tile([2 * g2, n], dt.float32, tag="sT2", name="sT2")
        out2 = [pp.tile([P, f], dt.float32, tag=f"o2_{i}", name=f"o2_{i}") for i in range(nb)]
        onesc = pp.tile([P, 1], dt.float32, tag="ones", name="ones")
        nc.vector.memset(onesc[:], 1.0)
        x1f = [pp.tile([P, c1], dt.bfloat16, tag=f"x1f{i}", name=f"x1f{i}") for i in range(nb)]

        # ================= layer 1 =================
        with tc.tile_pool(name="pA", bufs=1) as pA:
            x1pre = [pA.tile([P, c1], dt.float32, tag=f"x1p{i}", name=f"x1p{i}") for i in range(nb)]
            p1all = [pA.tile([P, h1 * fp1], dt.bfloat16, tag=f"p1_{i}", name=f"p1_{i}") for i in range(nb)]
            s1 = [pA.tile([P, 2 * h1], dt.float32, tag=f"s1_{i}", name=f"s1_{i}") for i in range(nb)]
            s1b = [pA.tile([P, 2 * h1], dt.float32, tag=f"s1b{i}", name=f"s1b{i}") for i in range(nb)]

            # ---- projection (p, s) ----
            with tc.tile_pool(name="pW1", bufs=1) as pW1, \
                 tc.tile_pool(name="psA", bufs=3, space="PSUM") as psA, \
                 tc.tile_pool(name="psT", bufs=2, space="PSUM") as psT:
                w1pt = []
                for k in range(kt1):
                    t = pW1.tile([P, c1 + 2 * h1], dt.bfloat16, tag=f"w1p{k}", name=f"w1p{k}")
                    nc.sync.dma_start(t[:], d["w1p"][k * P:(k + 1) * P, :])
                    w1pt.append(t)
                for ib in range(nb):
                    p1v = p1all[ib][:].rearrange("p (h f) -> p h f", f=fp1)
                    for hp in range(0, h1, 2):
                        po = psA.tile([P, 2 * f], dt.float32, tag="proj", name="proj")
                        for k in range(kt1):
                            nc.tensor.matmul(
                                po[:], xT[k][:, ib * P:(ib + 1) * P],
                                w1pt[k][:, hp * f:(hp + 2) * f],
                                start=(k == 0), stop=(k == kt1 - 1))
                        pov = po[:].rearrange("p (h f) -> p h f", f=f)
                        nc.vector.tensor_copy(p1v[:, hp:hp + 2, 0:f], pov[:])
                    po = psA.tile([P, 2 * h1], dt.float32, tag="projs", name="projs")
                    for k in range(kt1):
                        nc.tensor.matmul(po[:], xT[k][:, ib * P:(ib + 1) * P],
                                         w1pt[k][:, c1:c1 + 2 * h1],
                                         start=(k == 0), stop=(k == kt1 - 1))
                    nc.vector.tensor_copy(s1[ib][:], po[:])
                    nc.vector.tensor_scalar(out=s1b[ib][:], in0=po[:], scalar1=0.2,
                                            scalar2=None, op0=ALU.mult)
                    for h in range(h1):
                        nc.vector.memset(p1v[:, h, f:fp1], 1.0)
                    pt = psT.tile([2 * h1, P], dt.float32, tag="sT", name="sT")
                    nc.tensor.transpose(pt[:], s1[ib][:], idf[:])
                    nc.vector.tensor_copy(sT1[:, ib * P:(ib + 1) * P], pt[:])

            # ---- attention (writes x1pre head slices) ----
            _attention(tc, nc, cfg, has_mask, em_d,
                       heads=h1, p_tiles=p1all, s_tiles=s1, s02_tiles=s1b,
                       sT=sT1, dst=x1pre, dst_accum=False, nb=nb, f=f, fp1=fp1,
                       layer=1)

            # ---- skip projection (+b1 row) + ELU -> x1f ----
            with tc.tile_pool(name="pD", bufs=1) as pD, \
                 tc.tile_pool(name="pDe", bufs=2) as pDe, \
                 tc.tile_pool(name="psD", bufs=3, space="PSUM") as psD:
                w1st = []
                for k in range(kt1):
                    t = pD.tile([P, c1], dt.bfloat16, tag=f"w1s{k}", name=f"w1s{k}")
                    nc.sync.dma_start(t[:], d["w1s"][k * P:(k + 1) * P, :])
                    w1st.append(t)
                w1sb = pD.tile([1, c1], dt.bfloat16, tag="w1sb", name="w1sb")
                nc.sync.dma_start(w1sb[:], d["w1s"][din:din + 1, :])
                for ib in range(nb):
                    for cc in range(0, c1, 512):
                        po = psD.tile([P, 512], dt.float32, tag="skip", name="skip")
                        for k in range(kt1):
                            nc.tensor.matmul(po[:], xT[k][:, ib * P:(ib + 1) * P],
                                             w1st[k][:, cc:cc + 512],
                                             start=(k == 0), stop=False)
                        nc.tensor.matmul(po[:], xTones[:, ib * P:(ib + 1) * P],
                                         w1sb[:, cc:cc + 512],
                                         start=False, stop=True)
                        nc.vector.scalar_tensor_tensor(
                            out=x1pre[ib][:, cc:cc + 512], in0=po[:], scalar=0.0,
                            in1=x1pre[ib][:, cc:cc + 512], op0=ALU.add, op1=ALU.add)
                    # ELU in chunks of 1024
                    for cc in range(0, c1, 1024):
                        w = min(1024, c1 - cc)
                        sl = slice(cc, cc + w)
                        r = pDe.tile([P, w], dt.float32, tag="elur", name="elur")
                        nc.scalar.activation(r[:], x1pre[ib][:, sl], AF.Relu)
                        nc.vector.tensor_sub(x1pre[ib][:, sl], x1pre[ib][:, sl], r[:])
                        e = pDe.tile([P, w], dt.float32, tag="elue", name="elue")
                        nc.scalar.activation(e[:], x1pre[ib][:, sl], AF.Exp)
                        nc.vector.scalar_tensor_tensor(
                            out=x1f[ib][:, sl], in0=e[:], scalar=-1.0, in1=r[:],
                            op0=ALU.add, op1=ALU.add)

        # ================= transpose x1f -> x1fT =================
        with tc.tile_pool(name="pE", bufs=1) as pE:
            x1fT = [pE.tile([P, n], dt.bfloat16, tag=f"xT2_{c}", name=f"xT2_{c}") for c in range(ct)]
            with tc.tile_pool(name="psE", bufs=2, space="PSUM") as psE:
                for cb in range(ct):
                    for i0 in range(0, nb, 4):
                        nq = min(4, nb - i0)
                        po = psE.tile([P, nq * P], dt.bfloat16, tag="tr", name="tr")
                        for q in range(nq):
                            nc.tensor.transpose(
                                po[:, q * P:(q + 1) * P],
                                x1f[i0 + q][:, cb * P:(cb + 1) * P], idb[:])
                        nc.vector.tensor_copy(
                            x1fT[cb][:, i0 * P:(i0 + nq) * P], po[:])

            # ================= layer 2 =================
            with tc.tile_pool(name="pF", bufs=1) as pF:
                p2all = [pF.tile([P, g2 * fp1], dt.bfloat16, tag=f"p2_{i}", name=f"p2_{i}") for i in range(nb)]
                s2 = [pF.tile([P, 2 * g2], dt.float32, tag=f"s2_{i}", name=f"s2_{i}") for i in range(nb)]
                s2b = [pF.tile([P, 2 * g2], dt.float32, tag=f"s2b{i}", name=f"s2b{i}") for i in range(nb)]
                w2et = []
                for k in range(kt2):
                    t = pF.tile([P, w2cols], dt.bfloat16, tag=f"w2e{k}", name=f"w2e{k}")
                    nc.sync.dma_start(t[:], d["w2e"][k * P:(k + 1) * P, :])
                    w2et.append(t)

                with tc.tile_pool(name="psF", bufs=2, space="PSUM") as psF, \
                     tc.tile_pool(name="psT2", bufs=1, space="PSUM") as psT2:
                    for ib in range(nb):
                        p2v = p2all[ib][:].rearrange("p (h f) -> p h f", f=fp1)
                        for hp in range(0, g2, 2):
                            po = psF.tile([P, 2 * f], dt.float32, tag="proj2", name="proj2")
                            for k in range(kt2):
                                nc.tensor.matmul(
                                    po[:], x1fT[k][:, ib * P:(ib + 1) * P],
                                    w2et[k][:, hp * f:(hp + 2) * f],
                                    start=(k == 0), stop=(k == kt2 - 1))
                            pov = po[:].rearrange("p (h f) -> p h f", f=f)
                            nc.vector.tensor_copy(p2v[:, hp:hp + 2, 0:f], pov[:])
                        po = psF.tile([P, f], dt.float32, tag="skip2", name="skip2")
                        for k in range(kt2):
                            nc.tensor.matmul(po[:], x1fT[k][:, ib * P:(ib + 1) * P],
                                             w2et[k][:, g2 * f:(g2 + 1) * f],
                                             start=(k == 0), stop=(k == kt2 - 1))
                        nc.vector.tensor_copy(out2[ib][:], po[:])
                        po = psF.tile([P, 2 * g2], dt.float32, tag="proj2s", name="proj2s")
                        for k in range(kt2):
                            nc.tensor.matmul(po[:], x1fT[k][:, ib * P:(ib + 1) * P],
                                             w2et[k][:, (g2 + 1) * f:(g2 + 1) * f + 2 * g2],
                                             start=(k == 0), stop=(k == kt2 - 1))
                        nc.vector.tensor_copy(s2[ib][:], po[:])
                        nc.vector.tensor_scalar(out=s2b[ib][:], in0=po[:], scalar1=0.2,
                                                scalar2=None, op0=ALU.mult)
                        for h in range(g2):
                            nc.vector.memset(p2v[:, h, f:fp1], 1.0)
                        pt = psT2.tile([2 * g2, P], dt.float32, tag="sT2", name="sT2")
                        nc.tensor.transpose(pt[:], s2[ib][:], idf[:])
                        nc.vector.tensor_copy(sT2[:, ib * P:(ib + 1) * P], pt[:])

                _attention(tc, nc, cfg, has_mask, em_d,
                           heads=g2, p_tiles=p2all, s_tiles=s2, s02_tiles=s2b,
                           sT=sT2, dst=out2, dst_accum=True, nb=nb, f=f, fp1=fp1,
                           layer=2)

        # ================= pooling =================
        with tc.tile_pool(name="psH", bufs=1, space="PSUM") as psH, \
             tc.tile_pool(name="pH", bufs=1) as pH:
            pg = psH.tile([1, f], dt.float32, tag="pool", name="pool")
            for ib in range(nb):
                nc.tensor.matmul(pg[:], onesc[:], out2[ib][:],
                                 start=(ib == 0), stop=(ib == nb - 1))
            gout = pH.tile([1, f], dt.float32, tag="g", name="g")
            nc.vector.tensor_copy(gout[:], pg[:])
            nc.sync.dma_start(d["gpart"][:], gout[:])


def _attention(tc, nc, cfg, has_mask, em_d, *, heads, p_tiles, s_tiles,
               s02_tiles, sT, dst, dst_accum, nb, f, fp1, layer):
    """Dense masked-softmax attention for one layer.

    layer 1: dst[ib][:, h*f:(h+1)*f] = A_h @ p_h / r   (written per head)
    layer 2: dst[ib] += A_h @ p_h / r                  (accumulated over heads)
    """
    n = cfg["n"]
    with tc.tile_pool(name=f"att{layer}", bufs=1) as pa, \
         tc.tile_pool(name=f"sb{layer}", bufs=2) as pb, \
         tc.tile_pool(name=f"ex{layer}", bufs=2) as pe, \
         tc.tile_pool(name=f"em{layer}", bufs=1) as pm, \
         tc.tile_pool(name=f"ps{layer}", bufs=4, space="PSUM") as ps, \
         tc.tile_pool(name=f"rc{layer}", bufs=4) as prc:
        em_tiles = None
        if has_mask:
            em_tiles = []
            for jt in range(nb):
                em = pm.tile([P, n], dt.bfloat16, tag=f"em{jt}", name=f"em{jt}")
                nc.sync.dma_start(em[:], em_d[jt * P:(jt + 1) * P, :])
                em_tiles.append(em)
        p1v = [p_tiles[jt][:].rearrange("p (h f) -> p h f", f=fp1)
               for jt in range(nb)]
        for h in range(heads):
            srow = pb.tile([1, n], dt.float32, tag="srow", name="srow")
            nc.sync.dma_start(srow[:], sT[h:h + 1, :])
            sb = pb.tile([P, n], dt.float32, tag="sb", name="sb")
            nc.gpsimd.partition_broadcast(sb[:], srow[:])
            A_t = []
            for jt in range(nb):
                a1 = pe.tile([P, n], dt.bfloat16, tag="a1", name="a1")
                nc.scalar.activation(a1[:], sb[:], AF.Exp,
                                     bias=s_tiles[jt][:, heads + h:heads + h + 1])
                a2 = pe.tile([P, n], dt.bfloat16, tag="a2", name="a2")
                nc.scalar.activation(a2[:], sb[:], AF.Exp, scale=0.2,
                                     bias=s02_tiles[jt][:, heads + h:heads + h + 1])
                A = pa.tile([P, n], dt.bfloat16, tag=f"A{jt}", name=f"A{jt}")
                if has_mask:
                    am = pe.tile([P, n], dt.bfloat16, tag="am", name="am")
                    nc.vector.tensor_tensor(am[:], a1[:], a2[:], op=ALU.max)
                    nc.vector.tensor_tensor(A[:], am[:], em_tiles[jt][:], op=ALU.mult)
                else:
                    nc.vector.tensor_tensor(A[:], a1[:], a2[:], op=ALU.max)
                A_t.append(A)
            for ib in range(nb):
                po = ps.tile([P, fp1], dt.float32, tag="attn", name="attn")
                for jt in range(nb):
                    nc.tensor.matmul(po[:], A_t[jt][:, ib * P:(ib + 1) * P],
                                     p1v[jt][:, h, :],
                                     start=(jt == 0), stop=(jt == nb - 1))
                rc = prc.tile([P, 1], dt.float32, tag="rc", name="rc")
                nc.vector.reciprocal(rc[:], po[:, f:f + 1])
                if dst_accum:
                    nc.vector.scalar_tensor_tensor(
                        out=dst[ib][:], in0=po[:, 0:f], scalar=rc[:],
                        in1=dst[ib][:], op0=ALU.mult, op1=ALU.add)
                else:
                    nc.vector.tensor_scalar(
                        out=dst[ib][:, h * f:(h + 1) * f], in0=po[:, 0:f],
                        scalar1=rc[:], scalar2=None, op0=ALU.mult)


# ---------------------------------------------------------------------------
# host side
# ---------------------------------------------------------------------------

_COMPILED = {}


def _get_program(has_mask):
    key = bool(has_mask)
    if key not in _COMPILED:
        _COMPILED[key] = build_program(has_mask=key)
    return _COMPILED[key]


def make_host_inputs(features, attn_mask, W1, a_src1, a_tgt1, skip1, b1,
                     W2, a_src2, a_tgt2, skip2, has_mask, cfg=None):
    """Per-core input dicts (numpy)."""
    cfg = dict(CFG if cfg is None else cfg)
    n, h1, g2, f, din = cfg["n"], cfg["h1"], cfg["g2"], cfg["f"], cfg["din"]
    c1 = h1 * f
    ngrp = h1 // g2
    f32 = np.float32
    W1 = np.asarray(W1, f32); skip1 = np.asarray(skip1, f32)
    W2 = np.asarray(W2, f32); skip2 = np.asarray(skip2, f32)
    b1 = np.asarray(b1, f32)
    wsrc1 = np.einsum("dhf,hf->dh", W1.reshape(din, h1, f), np.asarray(a_src1, f32))
    wtgt1 = np.einsum("dhf,hf->dh", W1.reshape(din, h1, f), np.asarray(a_tgt1, f32))
    w1p = np.concatenate([W1, wsrc1, wtgt1], axis=1).astype(BF16)
    w1s = np.concatenate([skip1, b1.reshape(1, c1)], axis=0).astype(BF16)
    idf = np.eye(P, dtype=f32)
    idb = np.eye(P).astype(BF16)

    w2e_g = []
    for g in range(ngrp):
        cols = slice(g * g2 * f, (g + 1) * g2 * f)
        w2g = W2[:, cols]
        a_s = np.asarray(a_src2, f32)[g * g2:(g + 1) * g2]
        a_t = np.asarray(a_tgt2, f32)[g * g2:(g + 1) * g2]
        wsrc2 = np.einsum("dkf,kf->dk", w2g.reshape(c1, g2, f), a_s)
        wtgt2 = np.einsum("dkf,kf->dk", w2g.reshape(c1, g2, f), a_t)
        sk2sum = skip2[:, cols].reshape(c1, g2, f).sum(axis=1)
        w2e_g.append(np.concatenate([w2g, sk2sum, wsrc2, wtgt2], axis=1).astype(BF16))

    nbatch = np.asarray(features).shape[0]
    in_maps = []
    for c in range(nbatch * ngrp):
        b = c // ngrp
        g = c % ngrp
        xT = np.concatenate(
            [np.asarray(features[b], f32).T, np.ones((1, n), f32)], axis=0)
        m = dict(
            xT=np.ascontiguousarray(xT).astype(BF16),
            w1p=w1p, w1s=w1s, w2e=w2e_g[g], idf=idf, idb=idb,
        )
        if has_mask:
            mT = np.ascontiguousarray(np.asarray(attn_mask[b], f32).T)
            m["expmT"] = np.exp(np.maximum(mT, -80.0)).astype(BF16)
        in_maps.append(m)
    return in_maps


def finish_host(results, b2, Wc, bc, cfg=None):
    cfg = dict(CFG if cfg is None else cfg)
    n, h1, g2 = cfg["n"], cfg["h1"], cfg["g2"]
    ngrp = h1 // g2
    b2 = np.asarray(b2, np.float64)
    Wc = np.asarray(Wc, np.float64)
    bc = np.asarray(bc, np.float64)
    nbatch = len(results) // ngrp
    out = np.zeros((nbatch, Wc.shape[1]), np.float64)
    for b in range(nbatch):
        gsum = sum(results[b * ngrp + g]["gpart"][0].astype(np.float64)
                   for g in range(ngrp))
        gv = gsum / (h1 * n) + b2
        out[b] = gv @ Wc + bc
    return out.astype(np.float32)


def kernel(features, eigvects, attn_mask, W1, a_src1, a_tgt1, skip1, b1,
           W2, a_src2, a_tgt2, skip2, b2, Wc, bc):
    has_mask = bool(np.any(np.asarray(attn_mask)))
    nc = _get_program(has_mask)
    in_maps = make_host_inputs(features, attn_mask, W1, a_src1, a_tgt1, skip1,
                               b1, W2, a_src2, a_tgt2, skip2, has_mask)
    res = run_bass_kernel_spmd(nc, in_maps, list(range(NCORES)))
    return finish_host(res.results, b2, Wc, bc)
